# revision 1
# baseline (speedup 1.0000x reference)
"""AlphaEntmaxRouter (alpha=1.5) Trainium2 kernel.

Full inputs -> full output. Data-parallel over 8 NeuronCores (token dim
sharded 4096/core), weights replicated.

Per core:
  - x [4096, 2048] streamed in 8 blocks of 512 tokens (token t = p*32 + g).
    The SWDGE DMA casts fp32 -> fp16 in flight.
  - PE transposes the fp16 x tiles (transpose mode, 1 cyc/row) into PSUM,
    ACT copies them to SBUF, then a W-stationary fp16 matmul accumulates
    logits^T [64, 512] in fp32 PSUM over the 16 k-tiles. Bias + the 0.5
    scale are folded into the transposed weights / epilogue; PE re-transposes
    logits into s = 0.5*(x@W.T+b) laid out [128 part, 32 group, 64 expert].
  - entmax-1.5 tau solved by 6 Newton iterations on the convex decreasing
    f(tau) = sum_e relu(s_e - tau)^2 - 1 from tau0 = max(s)-1 (converges
    below the reference's own 25-step bisection error). Each eval: DVE
    subtract (stride-0 tau broadcast) + 2x-mode relu + segmented X-reduces;
    squares go to ACT during the streaming phase and to GPSIMD afterwards
    (GPSIMD generates the SWDGE load descriptors while streaming).
  - p = relu(s-tau)^2 normalized by its sum, DMA'd out.
  - Newton units sized [16,8,4,4] groups so early solver work overlaps the
    DMA stream and the post-stream tail stays short. A post-schedule pass
    (_legalize_waits) splits multi-wait instructions for this walrus build.
"""

import numpy as np

N_TOKENS = 32768
D = 2048
E = 64
N_CORES = 8
TOK_PER_CORE = N_TOKENS // N_CORES  # 4096
KT = D // 128  # 16 k-tiles
N_NEWTON = 6

_BUILT = None


def _register_custom_ops():
    """Register the three fused DVE ops this kernel needs (runtime extension
    of concourse.dve_ops.OPS; rows 1..16 are taken by stock ops, the 5-bit
    row field allows up to 0x1f)."""
    import numpy as np
    import concourse.dve_ops as dve_ops
    from concourse.dve_spec import Spec, Src0, Src1, relu, sq, Scan, lower, _has_src1, AluOp
    from concourse.dve_uop import DveOpSpec

    existing = {o.name for o in dve_ops.OPS}

    def flat(in0, in1):
        P = in0.shape[0]
        a = np.asarray(in0, np.float32).reshape(P, -1)
        b = np.ascontiguousarray(np.asarray(in1, np.float32)).reshape(P, -1)
        return a, b

    def ref_scanq(in0, in1, c0, c1, c2):
        a, b = flat(in0, in1)
        r = np.maximum(a - b, 0.0)
        return np.cumsum(r * r, axis=-1, dtype=np.float32)

    def ref_scanr(in0, in1, c0, c1, c2):
        a, b = flat(in0, in1)
        return np.cumsum(np.maximum(a - b, 0.0), axis=-1, dtype=np.float32)

    def ref_q(in0, in1, c0, c1, c2):
        a, b = flat(in0, in1)
        r = np.maximum(a - b, 0.0)
        return r * r

    defs = [
        ("ENTMAX_SCANQ_ANT", Scan(AluOp.ADD, sq(relu(Src0 - Src1))), ref_scanq),
        ("ENTMAX_SCANR_ANT", Scan(AluOp.ADD, relu(Src0 - Src1)), ref_scanr),
        ("ENTMAX_Q_ANT", sq(relu(Src0 - Src1)), ref_q),
    ]
    ops = []
    for name, body, ref in defs:
        if name in existing:
            ops.append(next(o for o in dve_ops.OPS if o.name == name))
            continue
        spec = Spec(body=body, reference=ref)
        row = 1 + len(dve_ops.OPS)
        assert row < 0x20
        shas = {}
        for ver in ("v3", "v4"):
            uops = lower(spec, ver=ver)
            shas[ver] = DveOpSpec(
                name=name, opcode=row, uops=uops, rd1_en=_has_src1(spec)
            ).sha(ver)
        op = dve_ops.DveOp(name, spec, subdim=False, uops_sha=shas)
        dve_ops.OPS.append(op)
        dve_ops.CUSTOM_DVE_SPECS[name] = spec
        dve_ops._SUB_OPCODE_FOR_NAME[name] = row
        ops.append(op)
    return ops



def _build():
    global _BUILT
    if _BUILT is not None:
        return _BUILT

    from contextlib import ExitStack

    import concourse.bass as bass
    import concourse.tile as tile
    from concourse import mybir
    from concourse.masks import make_identity

    f32 = mybir.dt.float32
    f16 = mybir.dt.float16
    OP = mybir.AluOpType
    AF = mybir.ActivationFunctionType
    AX = mybir.AxisListType

    BLOCKS = TOK_PER_CORE // 512  # 8
    GROUPS = TOK_PER_CORE // 128  # 32
    # bisection/newton work units (group ranges): big early units overlap the
    # streaming matmul phase; small late units shorten the tail after the
    # last DMA lands.
    UNITS = [(0, 16), (16, 24), (24, 28), (28, 32)]

    nc = bass.Bass("TRN2", debug=False)
    x = nc.dram_tensor("x", [TOK_PER_CORE, D], f32, kind="ExternalInput").ap()
    W = nc.dram_tensor("W", [E, D], f32, kind="ExternalInput").ap()
    b = nc.dram_tensor("b", [E, 1], f32, kind="ExternalInput").ap()
    out = nc.dram_tensor("out", [TOK_PER_CORE, E], f32, kind="ExternalOutput").ap()

    # token t = p*32 + g
    x_v = x.rearrange("(p g) d -> p g d", p=128)
    out_v = out.rearrange("(p g) e -> p g e", p=128)

    def bcast(ap2d, n):
        """[P, G] AP -> [P, G, n] stride-0 broadcast AP."""
        return bass.AP(tensor=ap2d.tensor, offset=ap2d.offset, ap=[*ap2d.ap, [0, n]])

    with tile.TileContext(nc) as tc, ExitStack() as ctx:
        singles = ctx.enter_context(tc.tile_pool(name="singles", bufs=1))
        xin_pool = ctx.enter_context(tc.tile_pool(name="xin", bufs=4))
        xt_pool = ctx.enter_context(tc.tile_pool(name="xt", bufs=4))
        lg_pool = ctx.enter_context(tc.tile_pool(name="lg", bufs=2))
        big_pool = ctx.enter_context(tc.tile_pool(name="big", bufs=4))
        sm_pool = ctx.enter_context(tc.tile_pool(name="sm", bufs=2))
        tp_psum = ctx.enter_context(tc.tile_pool(name="tp_ps", bufs=4, space="PSUM"))
        lg_psum = ctx.enter_context(tc.tile_pool(name="lg_ps", bufs=2, space="PSUM"))
        s_psum = ctx.enter_context(tc.tile_pool(name="s_ps", bufs=2, space="PSUM"))

        # ---- constants / weight prep -------------------------------------
        ident = singles.tile([128, 128], f32)
        make_identity(nc, ident)
        ident16 = singles.tile([128, 128], f16)
        nc.scalar.copy(out=ident16, in_=ident)

        w_nat = singles.tile([64, D], f32)
        nc.sync.dma_start(out=w_nat, in_=W)
        # Wait-absorber: the transpose-mode matmul's LDW struct only fits one
        # sync wait, so soak up the DMA-completion wait with a tiny standalone
        # bf16 ldweights; later PE instructions inherit it via program order.
        nc.tensor.ldweights(w_nat[:, 0:4].bitcast(mybir.dt.bfloat16))
        b_half = singles.tile([64, 1], f32)
        nc.sync.dma_start(out=b_half, in_=b)
        nc.scalar.mul(out=b_half, in_=b_half, mul=0.5)

        # wt[:, k, :] = 0.5 * W[:, 128k:128k+128].T   ([128 d, 64 e] per tile)
        wt = singles.tile([128, KT, E], f16)
        for k in range(KT):
            wps = s_psum.tile([128, E], f32, tag="sps")
            nc.tensor.matmul(
                wps,
                w_nat[:, k * 128 : (k + 1) * 128],
                ident[:64, :64],
                is_transpose=True,
            )
            nc.scalar.mul(out=wt[:, k, :], in_=wps, mul=0.5)

        # s[p, g, e] = 0.5 * (x @ W.T + b)[token p*32+g, e]
        s_sb = singles.tile([128, GROUPS, E], f32)

        # ---- streaming matmul phase --------------------------------------
        for blk in range(BLOCKS):
            xin = xin_pool.tile([128, 4, D], f16, tag="xin")
            # SWDGE DMA casts fp32 -> fp16 in flight (HWDGE cannot);
            # halves PE transpose cycles and xT copy bytes downstream.
            nc.gpsimd.dma_start(out=xin, in_=x_v[:, 4 * blk : 4 * blk + 4, :])
            nc.tensor.ldweights(xin[:, 0, 0:4].bitcast(mybir.dt.bfloat16))

            lg_ps = lg_psum.tile([64, 512], f32, tag="lgps")
            for kg in range(KT // 2):  # pairs of k-tiles share a PSUM tile
                tp = tp_psum.tile([128, 2, 512], f16, tag="tp")
                for i in range(2):
                    k = 2 * kg + i
                    for ch in range(4):
                        # [128 tok, 128 d] -> [128 d, 128 tok]
                        nc.tensor.matmul(
                            tp[:, i, ch * 128 : (ch + 1) * 128],
                            xin[:, ch, k * 128 : (k + 1) * 128],
                            ident16,
                            is_transpose=True,
                            skip_group_check=True,
                        )
                xt = xt_pool.tile([128, 2, 512], f16, tag="xt")
                nc.scalar.copy(out=xt, in_=tp)
                for i in range(2):
                    k = 2 * kg + i
                    nc.tensor.matmul(
                        lg_ps,
                        wt[:, k, :],
                        xt[:, i, :],
                        start=(k == 0),
                        stop=(k == KT - 1),
                    )
            # epilogue: add 0.5*b (per-partition = per-expert here)
            lg_sb = lg_pool.tile([64, 512], f32, tag="lgsb")
            nc.scalar.activation(
                out=lg_sb, in_=lg_ps, func=AF.Identity, bias=b_half, scale=1.0
            )
            nc.tensor.ldweights(lg_sb[:, 0:4].bitcast(mybir.dt.bfloat16))
            # de-transpose [64, 512] -> 4x [128, 64] into s
            for ch in range(4):
                sps = s_psum.tile([128, E], f32, tag="sps")
                nc.tensor.matmul(
                    sps,
                    lg_sb[:, ch * 128 : (ch + 1) * 128],
                    ident[:64, :64],
                    is_transpose=True,
                )
                nc.vector.tensor_copy(out=s_sb[:, 4 * blk + ch, :], in_=sps)

        # ---- entmax tau solve + output, per unit -------------------------
        def tt(o, a, bb, op):
            nc.vector.tensor_tensor(out=o, in0=a, in1=bb, op=op)

        for g0, g1 in UNITS:
            G = g1 - g0
            sv = s_sb[:, g0:g1, :]

            def sm(tag):
                return sm_pool.tile([128, G], f32, name=f"{tag}{g0}", tag=f"{tag}{g0}")

            mx = sm("mx")
            nc.vector.tensor_reduce(out=mx, in_=sv, axis=AX.X, op=OP.max)
            tau = sm("tau")
            nc.vector.tensor_scalar_add(out=tau, in0=mx, scalar1=-1.0)
            taub = bcast(tau, E)

            d = big_pool.tile([128, G, E], f32, name=f"d{g0}", tag="d", bufs=2)
            r = big_pool.tile([128, G, E], f32, name=f"r{g0}", tag="r", bufs=2)
            q = big_pool.tile([128, G, E], f32, name=f"q{g0}", tag="q", bufs=2)
            fq, fr, inv, stp = sm("fq"), sm("fr"), sm("inv"), sm("stp")

            def feval():
                tt(d, sv, taub, OP.subtract)     # d = s - tau
                nc.vector.tensor_scalar_max(out=r, in0=d, scalar1=0.0)  # relu
                # Early units overlap the streaming phase, whose x-loads are
                # SWDGE DMAs (descriptors generated on the GPSIMD Q7s) - keep
                # GPSIMD free then and square on ACT; tail units run after the
                # last load, so use the otherwise-idle GPSIMD.
                if g0 < 24:
                    nc.scalar.square(q, r)
                else:
                    nc.gpsimd.tensor_tensor(out=q, in0=r, in1=r, op=OP.mult)
                nc.vector.tensor_reduce(out=fq, in_=q, axis=AX.X, op=OP.add)

            for _ in range(N_NEWTON):
                feval()
                nc.vector.tensor_reduce(out=fr, in_=r, axis=AX.X, op=OP.add)
                # tau += (fq - 1) / (2 fr)
                nc.vector.reciprocal(out=inv, in_=fr)
                nc.vector.tensor_scalar(
                    out=fq, in0=fq, scalar1=-1.0, scalar2=0.5, op0=OP.add, op1=OP.mult
                )
                tt(stp, fq, inv, OP.mult)
                tt(tau, tau, stp, OP.add)

            # final: p = q / sum(q)
            feval()
            rcp = sm("rcp")
            nc.vector.reciprocal(out=rcp, in_=fq)
            pn = big_pool.tile([128, G, E], f32, name=f"pn{g0}", tag="pn", bufs=2)
            tt(pn, q, bcast(rcp, E), OP.mult)
            nc.sync.dma_start(out=out_v[:, g0:g1, :], in_=pn)

    _legalize_waits(nc)

    _BUILT = nc
    return nc


def _legalize_waits(nc):
    # Walrus codegen rejects instructions whose ISA struct lacks slots for
    # all the sync waits Tile attached (most structs fit only one). Legalize:
    # cap every instruction at one wait and hoist the extras onto same-engine
    # carrier InstDrains placed just before (drains carry sync_info in Tile's
    # own barriers, ~12ns each).
    from concourse import mybir

    ndrain = 0
    for fn in nc.m.functions:
        for blk in fn.blocks:
            new_insts = []
            for inst in blk.instructions:
                si = inst.sync_info
                if si is not None and si.on_wait and len(si.on_wait) > 1:
                    for w in list(si.on_wait)[:-1]:
                        d = mybir.InstDrain(
                            name=f"{inst.name}-wsplit{ndrain}",
                            ins=[],
                            outs=[],
                            bass_is_fusable=False,
                        )
                        ndrain += 1
                        d.engine = inst.engine
                        d.sync_info = mybir.SyncInfo(on_wait=[w], on_update=[])
                        new_insts.append(d)
                    inst.sync_info = mybir.SyncInfo(
                        on_wait=[si.on_wait[-1]], on_update=si.on_update
                    )
                new_insts.append(inst)
            blk.instructions = new_insts


def _run(x, W, b, trace=False):
    from concourse.bass_utils import run_bass_kernel_spmd

    nc = _build()
    x = np.ascontiguousarray(x, dtype=np.float32)
    W = np.ascontiguousarray(W, dtype=np.float32)
    b2 = np.ascontiguousarray(np.asarray(b, dtype=np.float32).reshape(E, 1))
    in_maps = [
        {
            "x": x[c * TOK_PER_CORE : (c + 1) * TOK_PER_CORE],
            "W": W,
            "b": b2,
        }
        for c in range(N_CORES)
    ]
    res = run_bass_kernel_spmd(nc, in_maps, core_ids=list(range(N_CORES)), trace=trace)
    full = np.concatenate([r["out"] for r in res.results], axis=0)
    return full, res


def kernel(x, W, b):
    full, _ = _run(x, W, b, trace=False)
    return full



# revision 2
# speedup vs baseline: 264.5651x; 264.5651x over previous
"""AlphaEntmaxRouter (alpha=1.5) Trainium2 kernel.

Full inputs -> full output. Data-parallel over 8 NeuronCores (token dim
sharded 4096/core), weights replicated.

Per core:
  - x [4096, 2048] fp32 streamed in 8 blocks of 512 tokens (token
    t = p*32 + g) via plain HWDGE DMA (4 MiB per dma_start).
  - ACT casts each block to fp16; PE transposes the fp16 tiles (transpose
    mode, 1 cyc/row) into PSUM and DVE evacuates them to SBUF in 2x mode.
  - W-stationary fp16 matmul accumulates logits^T [64, 512] in fp32 PSUM
    over the 16 k-tiles. Bias + the 0.5 scale are folded into the
    transposed weights / epilogue; PE re-transposes logits into
    s = 0.5*(x@W.T+b) laid out [128 part, 32 group, 64 expert].
  - entmax-1.5 tau solved by 6 Newton iterations on the convex decreasing
    f(tau) = sum_e relu(s_e - tau)^2 - 1 from tau0 = max(s)-1 (converges
    below the reference's own 25-step bisection error). Each eval: DVE
    subtract (stride-0 tau broadcast) + relu + ACT square + segmented
    X-reduce.
  - p = relu(s-tau)^2 normalized by its sum, DMA'd out.
  - Newton units sized [16,8,4,4] groups so early solver work overlaps the
    DMA stream and the post-stream tail stays short. A post-schedule pass
    (_legalize_waits) splits multi-wait instructions for this walrus build.
"""

import numpy as np

N_TOKENS = 32768
D = 2048
E = 64
N_CORES = 8
TOK_PER_CORE = N_TOKENS // N_CORES  # 4096
KT = D // 128  # 16 k-tiles
N_NEWTON = 6

_BUILT = None


def _build():
    global _BUILT
    if _BUILT is not None:
        return _BUILT

    from contextlib import ExitStack

    import concourse.bass as bass
    import concourse.tile as tile
    from concourse import mybir
    from concourse.masks import make_identity

    f32 = mybir.dt.float32
    f16 = mybir.dt.float16
    OP = mybir.AluOpType
    AF = mybir.ActivationFunctionType
    AX = mybir.AxisListType

    BLOCKS = TOK_PER_CORE // 512  # 8
    GROUPS = TOK_PER_CORE // 128  # 32
    # newton work units (group ranges): big early units overlap the
    # streaming matmul phase; small late units shorten the tail after the
    # last block lands.
    UNITS = [(0, 16), (16, 24), (24, 28), (28, 32)]

    nc = bass.Bass("TRN2", debug=False)
    x = nc.dram_tensor("x", [TOK_PER_CORE, D], f32, kind="ExternalInput").ap()
    W = nc.dram_tensor("W", [E, D], f32, kind="ExternalInput").ap()
    b = nc.dram_tensor("b", [E, 1], f32, kind="ExternalInput").ap()
    out = nc.dram_tensor("out", [TOK_PER_CORE, E], f32, kind="ExternalOutput").ap()

    # token t = p*32 + g
    x_v = x.rearrange("(p g) d -> p g d", p=128)
    out_v = out.rearrange("(p g) e -> p g e", p=128)

    def bcast(ap2d, n):
        """[P, G] AP -> [P, G, n] stride-0 broadcast AP."""
        return bass.AP(tensor=ap2d.tensor, offset=ap2d.offset, ap=[*ap2d.ap, [0, n]])

    with tile.TileContext(nc) as tc, ExitStack() as ctx:
        singles = ctx.enter_context(tc.tile_pool(name="singles", bufs=1))
        xin_pool = ctx.enter_context(tc.tile_pool(name="xin", bufs=2))
        xb_pool = ctx.enter_context(tc.tile_pool(name="xb", bufs=2))
        xt_pool = ctx.enter_context(tc.tile_pool(name="xt", bufs=2))
        lg_pool = ctx.enter_context(tc.tile_pool(name="lg", bufs=2))
        big_pool = ctx.enter_context(tc.tile_pool(name="big", bufs=4))
        sm_pool = ctx.enter_context(tc.tile_pool(name="sm", bufs=2))
        tp_psum = ctx.enter_context(tc.tile_pool(name="tp_ps", bufs=4, space="PSUM"))
        lg_psum = ctx.enter_context(tc.tile_pool(name="lg_ps", bufs=2, space="PSUM"))
        s_psum = ctx.enter_context(tc.tile_pool(name="s_ps", bufs=2, space="PSUM"))

        # ---- constants / weight prep -------------------------------------
        ident = singles.tile([128, 128], f32)
        make_identity(nc, ident)
        ident16 = singles.tile([128, 128], f16)
        nc.scalar.copy(out=ident16, in_=ident)

        w_nat = singles.tile([64, D], f32)
        nc.sync.dma_start(out=w_nat, in_=W)
        # Wait-absorber: the transpose-mode matmul's LDW struct only fits one
        # sync wait, so soak up the DMA-completion wait with a tiny standalone
        # bf16 ldweights; later PE instructions inherit it via program order.
        nc.tensor.ldweights(w_nat[:, 0:4].bitcast(mybir.dt.bfloat16))
        b_half = singles.tile([64, 1], f32)
        nc.sync.dma_start(out=b_half, in_=b)
        nc.scalar.mul(out=b_half, in_=b_half, mul=0.5)

        # wt[:, k, :] = 0.5 * W[:, 128k:128k+128].T   ([128 d, 64 e] per tile)
        wt = singles.tile([128, KT, E], f16)
        for k in range(KT):
            wps = s_psum.tile([128, E], f32, tag="sps")
            nc.tensor.matmul(
                wps,
                w_nat[:, k * 128 : (k + 1) * 128],
                ident[:64, :64],
                is_transpose=True,
            )
            nc.scalar.mul(out=wt[:, k, :], in_=wps, mul=0.5)

        # s[p, g, e] = 0.5 * (x @ W.T + b)[token p*32+g, e]
        s_sb = singles.tile([128, GROUPS, E], f32)

        # ---- streaming matmul phase --------------------------------------
        for blk in range(BLOCKS):
            xin = xin_pool.tile([128, 4, D], f32, tag="xin")
            nc.sync.dma_start(out=xin, in_=x_v[:, 4 * blk : 4 * blk + 4, :])
            # fp32 -> fp16 on ACT: halves PE transpose cycles and lets the
            # PSUM evacuation below run in DVE 2x mode.
            xb = xb_pool.tile([128, 4, D], f16, tag="xb")
            nc.scalar.copy(out=xb, in_=xin)

            xt = xt_pool.tile([128, KT, 512], f16, tag="xt")
            for g in range(4):
                for kh in range(2):  # halves of the k range share a PSUM bank
                    tp = tp_psum.tile([128, 8, 128], f16, tag="tp")
                    for i in range(8):
                        k = 8 * kh + i
                        # [128 tok, 128 d] -> [128 d, 128 tok]
                        nc.tensor.matmul(
                            tp[:, i, :],
                            xb[:, g, k * 128 : (k + 1) * 128],
                            ident16,
                            is_transpose=True,
                            skip_group_check=True,
                        )
                    nc.vector.tensor_copy(
                        out=xt[:, 8 * kh : 8 * kh + 8, 128 * g : 128 * g + 128],
                        in_=tp,
                    )

            lg_ps = lg_psum.tile([64, 512], f32, tag="lgps")
            for k in range(KT):
                nc.tensor.matmul(
                    lg_ps,
                    wt[:, k, :],
                    xt[:, k, :],
                    start=(k == 0),
                    stop=(k == KT - 1),
                )
            # epilogue: add 0.5*b (per-partition = per-expert here)
            lg_sb = lg_pool.tile([64, 512], f32, tag="lgsb")
            nc.scalar.activation(
                out=lg_sb, in_=lg_ps, func=AF.Identity, bias=b_half, scale=1.0
            )
            nc.tensor.ldweights(lg_sb[:, 0:4].bitcast(mybir.dt.bfloat16))
            # de-transpose [64, 512] -> 4x [128, 64] into s
            for ch in range(4):
                sps = s_psum.tile([128, E], f32, tag="sps")
                nc.tensor.matmul(
                    sps,
                    lg_sb[:, ch * 128 : (ch + 1) * 128],
                    ident[:64, :64],
                    is_transpose=True,
                )
                nc.vector.tensor_copy(out=s_sb[:, 4 * blk + ch, :], in_=sps)

        # ---- entmax tau solve + output, per unit -------------------------
        def tt(o, a, bb, op):
            nc.vector.tensor_tensor(out=o, in0=a, in1=bb, op=op)

        for g0, g1 in UNITS:
            G = g1 - g0
            sv = s_sb[:, g0:g1, :]

            def sm(tag):
                return sm_pool.tile([128, G], f32, name=f"{tag}{g0}", tag=f"{tag}{g0}")

            mx = sm("mx")
            nc.vector.tensor_reduce(out=mx, in_=sv, axis=AX.X, op=OP.max)
            tau = sm("tau")
            nc.vector.tensor_scalar_add(out=tau, in0=mx, scalar1=-1.0)
            taub = bcast(tau, E)

            d = big_pool.tile([128, G, E], f32, name=f"d{g0}", tag="d", bufs=2)
            r = big_pool.tile([128, G, E], f32, name=f"r{g0}", tag="r", bufs=2)
            q = big_pool.tile([128, G, E], f32, name=f"q{g0}", tag="q", bufs=2)
            fq, fr, inv, stp = sm("fq"), sm("fr"), sm("inv"), sm("stp")

            def feval():
                tt(d, sv, taub, OP.subtract)     # d = s - tau
                nc.vector.tensor_scalar_max(out=r, in0=d, scalar1=0.0)  # relu
                nc.scalar.square(q, r)
                nc.vector.tensor_reduce(out=fq, in_=q, axis=AX.X, op=OP.add)

            for _ in range(N_NEWTON):
                feval()
                nc.vector.tensor_reduce(out=fr, in_=r, axis=AX.X, op=OP.add)
                # tau += (fq - 1) / (2 fr)
                nc.vector.reciprocal(out=inv, in_=fr)
                nc.vector.tensor_scalar(
                    out=fq, in0=fq, scalar1=-1.0, scalar2=0.5, op0=OP.add, op1=OP.mult
                )
                tt(stp, fq, inv, OP.mult)
                tt(tau, tau, stp, OP.add)

            # final: p = q / sum(q)
            feval()
            rcp = sm("rcp")
            nc.vector.reciprocal(out=rcp, in_=fq)
            pn = big_pool.tile([128, G, E], f32, name=f"pn{g0}", tag="pn", bufs=2)
            tt(pn, q, bcast(rcp, E), OP.mult)
            nc.sync.dma_start(out=out_v[:, g0:g1, :], in_=pn)

    _legalize_waits(nc)

    _BUILT = nc
    return nc


def _legalize_waits(nc):
    # Walrus codegen rejects instructions whose ISA struct lacks slots for
    # all the sync waits Tile attached (most structs fit only one). Legalize:
    # cap every instruction at one wait and hoist the extras onto same-engine
    # carrier InstDrains placed just before (drains carry sync_info in Tile's
    # own barriers, ~12ns each).
    from concourse import mybir

    ndrain = 0
    for fn in nc.m.functions:
        for blk in fn.blocks:
            new_insts = []
            for inst in blk.instructions:
                si = inst.sync_info
                if si is not None and si.on_wait and len(si.on_wait) > 1:
                    for w in list(si.on_wait)[:-1]:
                        d = mybir.InstDrain(
                            name=f"{inst.name}-wsplit{ndrain}",
                            ins=[],
                            outs=[],
                            bass_is_fusable=False,
                        )
                        ndrain += 1
                        d.engine = inst.engine
                        d.sync_info = mybir.SyncInfo(on_wait=[w], on_update=[])
                        new_insts.append(d)
                    inst.sync_info = mybir.SyncInfo(
                        on_wait=[si.on_wait[-1]], on_update=si.on_update
                    )
                new_insts.append(inst)
            blk.instructions = new_insts


def _run(x, W, b, trace=False):
    from concourse.bass_utils import run_bass_kernel_spmd

    nc = _build()
    x = np.ascontiguousarray(x, dtype=np.float32)
    W = np.ascontiguousarray(W, dtype=np.float32)
    b2 = np.ascontiguousarray(np.asarray(b, dtype=np.float32).reshape(E, 1))
    in_maps = [
        {
            "x": x[c * TOK_PER_CORE : (c + 1) * TOK_PER_CORE],
            "W": W,
            "b": b2,
        }
        for c in range(N_CORES)
    ]
    res = run_bass_kernel_spmd(nc, in_maps, core_ids=list(range(N_CORES)), trace=trace)
    full = np.concatenate([r["out"] for r in res.results], axis=0)
    return full, res


def kernel(x, W, b):
    full, _ = _run(x, W, b, trace=False)
    return full
